# revision 1
# baseline (speedup 1.0000x reference)
"""AffinityPropagate Trainium2 kernel.

Math: the reference iterates fm <- fm + G@fm five times with a per-pixel
5x5 gate matrix G (softmax over groups of 5 guidance channels). This is
linear, so the result is out = (I+G)^5 @ fm -- computed here as one
per-pixel 5x5 matrix power (A2=A*A, A4=A2*A2, M=A4*A) followed by a
single 5x5 @ 5x64 per-pixel apply.

Sharding: pure data parallel over 8 cores; core s takes batch b=s//2,
rows h in [ (s%2)*48, (s%2)*48+48 ) -- 15360 pixels per core.

On-chip layout: pixels are split [128 partitions x 120 free]; gate
channels (25) and feature channels (64) live as separate free-dim
blocks, so all elementwise ops run with large free dims. Per-pixel 5x5
matrix products are fused into 9 big tensor ops each via step-0
broadcast access patterns.
"""

import sys

sys.path.insert(0, "/opt/trn_rl_repo")

import numpy as np

import concourse.bacc as bacc
import concourse.mybir as mybir
import concourse.tile as tile
from concourse.bass_utils import run_bass_kernel_spmd

B, C, H, W = 4, 64, 96, 320
K = 5
NCORES = 8
HSH = H // 2  # 48 rows per shard
NPIX = HSH * W  # 15360 pixels per core
P = 128
F = NPIX // P  # 120 free columns
CCH = 16  # feature channels per apply chunk
NCH = C // CCH
FD = CCH * F

_f32 = mybir.dt.float32
_mult = mybir.AluOpType.mult
_add = mybir.AluOpType.add

_cache = {}


def _build():
    nc = bacc.Bacc(None)
    g = nc.declare_dram_parameter("g", [25, NPIX], _f32, isOutput=False)
    fm = nc.declare_dram_parameter("fm", [K, C, NPIX], _f32, isOutput=False)
    out = nc.declare_dram_parameter("out", [K, C, NPIX], _f32, isOutput=True)

    def v4(t):  # [P, 25F] tile -> [P, K, K, F]
        return t[:].rearrange("p (k j f) -> p k j f", k=K, j=K)

    with tile.TileContext(nc) as tc:
        with (
            tc.tile_pool(name="gates", bufs=1) as gp,
            tc.tile_pool(name="tmps", bufs=2) as tp,
            tc.tile_pool(name="fmp", bufs=2) as fp,
            tc.tile_pool(name="outp", bufs=3) as op_,
        ):
            # --- gates: E = exp(g) ---
            GR = gp.tile([P, 25 * F], _f32, tag="bufA")
            nc.sync.dma_start(
                out=GR[:].rearrange("p (kj f) -> p kj f", kj=25),
                in_=g[:].rearrange("kj (r f) -> r kj f", r=P),
            )
            GE = gp.tile([P, 25 * F], _f32, tag="bufB")
            nc.scalar.activation(GE[:], GR[:], mybir.ActivationFunctionType.Exp)

            # --- softmax denominators and normalize: A = E/s (+I) ---
            SS = gp.tile([P, K * F], _f32, tag="ss")
            nc.vector.tensor_reduce(
                SS[:].rearrange("p (k f) -> p k f", k=K),
                GE[:].rearrange("p (k j f) -> p k f j", k=K, j=K),
                axis=mybir.AxisListType.X,
                op=_add,
            )
            RR = gp.tile([P, K * F], _f32, tag="rr")
            nc.vector.reciprocal(RR[:], SS[:])
            AA = gp.tile([P, 25 * F], _f32, tag="aa")
            rrb = (
                RR[:]
                .rearrange("p (k f) -> p k f", k=K)
                .unsqueeze(2)
                .broadcast_to((P, K, K, F))
            )
            nc.vector.tensor_tensor(v4(AA), v4(GE), rrb, _mult)
            for k in range(K):
                sl = AA[:, (k * K + k) * F : (k * K + k + 1) * F]
                nc.vector.tensor_scalar_add(sl, sl, 1.0)

            # --- per-pixel 5x5 matrix power M = A^5 ---
            def matmul5(dst, x, y):
                d4, x4, y4 = v4(dst), v4(x), v4(y)
                for l in range(K):
                    i0 = x4[:, :, l : l + 1, :].broadcast_to((P, K, K, F))
                    i1 = y4[:, l : l + 1, :, :].broadcast_to((P, K, K, F))
                    if l == 0:
                        nc.vector.tensor_tensor(d4, i0, i1, _mult)
                    else:
                        t = tp.tile([P, 25 * F], _f32, tag="mm_tmp")
                        nc.vector.tensor_tensor(v4(t), i0, i1, _mult)
                        nc.vector.tensor_tensor(dst[:], dst[:], t[:], _add)

            A2 = gp.tile([P, 25 * F], _f32, tag="bufA")
            matmul5(A2, AA, AA)
            A4 = gp.tile([P, 25 * F], _f32, tag="bufB")
            matmul5(A4, A2, A2)
            MM = gp.tile([P, 25 * F], _f32, tag="mm")
            matmul5(MM, A4, AA)

            # --- apply: out[k] = sum_j M[k,j] * fm[j], chunked over c ---
            for cc in range(NCH):
                c0 = cc * CCH
                fms = []
                for j in range(K):
                    t = fp.tile([P, FD], _f32, tag=f"fm{j}")
                    nc.sync.dma_start(
                        out=t[:].rearrange("p (c f) -> p c f", c=CCH),
                        in_=fm[j].rearrange("c (r f) -> r c f", r=P)[
                            :, c0 : c0 + CCH, :
                        ],
                    )
                    fms.append(t)
                for k in range(K):
                    ot = op_.tile([P, FD], _f32, tag="out")
                    o3 = ot[:].rearrange("p (c f) -> p c f", c=CCH)
                    for j in range(K):
                        mv = (
                            MM[:, (k * K + j) * F : (k * K + j + 1) * F]
                            .unsqueeze(1)
                            .broadcast_to((P, CCH, F))
                        )
                        f3 = fms[j][:].rearrange("p (c f) -> p c f", c=CCH)
                        if j == 0:
                            nc.vector.tensor_tensor(o3, f3, mv, _mult)
                        else:
                            t2 = tp.tile([P, FD], _f32, tag="ap_tmp")
                            nc.vector.tensor_tensor(
                                t2[:].rearrange("p (c f) -> p c f", c=CCH), f3, mv, _mult
                            )
                            nc.vector.tensor_tensor(ot[:], ot[:], t2[:], _add)
                    nc.sync.dma_start(
                        out=out[k].rearrange("c (r f) -> r c f", r=P)[
                            :, c0 : c0 + CCH, :
                        ],
                        in_=o3,
                    )
    nc.finalize()
    return nc


def _get_nc():
    if "nc" not in _cache:
        _cache["nc"] = _build()
    return _cache["nc"]


def kernel(guidance, fm0, fm1, fm2, fm3, fm4):
    nc = _get_nc()
    fms = [np.asarray(x, dtype=np.float32) for x in (fm0, fm1, fm2, fm3, fm4)]
    guidance = np.asarray(guidance, dtype=np.float32)

    in_maps = []
    for s in range(NCORES):
        b, h0 = s // 2, (s % 2) * HSH
        g_s = np.ascontiguousarray(
            guidance[b, :, h0 : h0 + HSH, :].reshape(25, NPIX)
        )
        fm_s = np.empty((K, C, NPIX), dtype=np.float32)
        for j in range(K):
            fm_s[j] = fms[j][b, :, h0 : h0 + HSH, :].reshape(C, NPIX)
        in_maps.append({"g": g_s, "fm": fm_s})

    res = run_bass_kernel_spmd(nc, in_maps, list(range(NCORES)))

    full = np.empty((K, B, C, H, W), dtype=np.float32)
    for s in range(NCORES):
        b, h0 = s // 2, (s % 2) * HSH
        full[:, b, :, h0 : h0 + HSH, :] = res.results[s]["out"].reshape(
            K, C, HSH, W
        )
    return full


# revision 2
# speedup vs baseline: 1.8662x; 1.8662x over previous
"""AffinityPropagate Trainium2 kernel.

Math: the reference iterates fm <- fm + G@fm five times with a per-pixel
5x5 gate matrix G (softmax over groups of 5 guidance channels). This is
linear, so the result is out = (I+G)^5 @ fm -- computed here as one
per-pixel 5x5 matrix power (A2=A*A, A4=A2*A2, M=A4*A) followed by a
single 5x5 @ 5x64 per-pixel apply.

Sharding: pure data parallel over 8 cores; core s takes batch b=s//2,
rows h in [ (s%2)*48, (s%2)*48+48 ) -- 15360 pixels per core.

On-chip layout: pixels are split [128 partitions x 120 free]; gate
channels (25) and feature channels (64) live as separate free-dim
blocks, so all elementwise ops run with large free dims. Per-pixel 5x5
matrix products are fused into 9 big tensor ops each via step-0
broadcast access patterns. Gates/softmax run in fp32; the matrix power
and the feature apply run in bf16 (DVE 2x mode), with fm/out DRAM
traffic in bf16. DRAM layouts are partition-major so every DMA row is
a multi-KB contiguous run.
"""

import sys

sys.path.insert(0, "/opt/trn_rl_repo")

import ml_dtypes
import numpy as np

import concourse.bacc as bacc
import concourse.mybir as mybir
import concourse.tile as tile
from concourse.bass_utils import run_bass_kernel_spmd

B, C, H, W = 4, 64, 96, 320
K = 5
NCORES = 8
HSH = H // 2  # 48 rows per shard
NPIX = HSH * W  # 15360 pixels per core
P = 128
F = NPIX // P  # 120 free columns
CCH = 16  # feature channels per apply chunk
NCH = C // CCH
FD = CCH * F

_f32 = mybir.dt.float32
_bf16 = mybir.dt.bfloat16
_npbf16 = ml_dtypes.bfloat16
_mult = mybir.AluOpType.mult
_add = mybir.AluOpType.add

_cache = {}


def _build():
    nc = bacc.Bacc(None)
    g = nc.declare_dram_parameter("g", [P, 25, F], _f32, isOutput=False)
    fm = nc.declare_dram_parameter("fm", [K, P, C, F], _bf16, isOutput=False)
    out = nc.declare_dram_parameter("out", [K, P, C, F], _bf16, isOutput=True)

    def v4(t):  # [P, 25F] tile -> [P, K, K, F]
        return t[:].rearrange("p (k j f) -> p k j f", k=K, j=K)

    with tile.TileContext(nc) as tc:
        with (
            tc.tile_pool(name="gates", bufs=1) as gp,
            tc.tile_pool(name="tmps", bufs=2) as tp,
            tc.tile_pool(name="fmp", bufs=3) as fp,
            tc.tile_pool(name="outp", bufs=3) as op_,
        ):
            # --- gates: E = exp(g) ---
            GR = gp.tile([P, 25 * F], _f32, tag="bufA")
            nc.sync.dma_start(
                out=GR[:].rearrange("p (kj f) -> p kj f", kj=25),
                in_=g[:],
            )
            GE = gp.tile([P, 25 * F], _f32, tag="bufB")
            nc.scalar.activation(GE[:], GR[:], mybir.ActivationFunctionType.Exp)

            # --- softmax denominators and normalize: A = E/s (+I) ---
            SS = gp.tile([P, K * F], _f32, tag="ss")
            nc.vector.tensor_reduce(
                SS[:].rearrange("p (k f) -> p k f", k=K),
                GE[:].rearrange("p (k j f) -> p k f j", k=K, j=K),
                axis=mybir.AxisListType.X,
                op=_add,
            )
            RR = gp.tile([P, K * F], _f32, tag="rr")
            nc.vector.reciprocal(RR[:], SS[:])
            AA = gp.tile([P, 25 * F], _f32, tag="aa")
            rrb = (
                RR[:]
                .rearrange("p (k f) -> p k f", k=K)
                .unsqueeze(2)
                .broadcast_to((P, K, K, F))
            )
            nc.vector.tensor_tensor(v4(AA), v4(GE), rrb, _mult)
            for k in range(K):
                sl = AA[:, (k * K + k) * F : (k * K + k + 1) * F]
                nc.vector.tensor_scalar_add(sl, sl, 1.0)
            # cast to bf16 for the matrix power (on the idle ACT engine)
            Ab = gp.tile([P, 25 * F], _bf16, tag="ab")
            nc.scalar.copy(Ab[:], AA[:])

            # --- per-pixel 5x5 matrix power M = A^5 (bf16) ---
            def matmul5(dst, x, y):
                d4, x4, y4 = v4(dst), v4(x), v4(y)
                for l in range(K):
                    i0 = x4[:, :, l : l + 1, :].broadcast_to((P, K, K, F))
                    i1 = y4[:, l : l + 1, :, :].broadcast_to((P, K, K, F))
                    if l == 0:
                        nc.vector.tensor_tensor(d4, i0, i1, _mult)
                    else:
                        t = tp.tile([P, 25 * F], _bf16, tag="mm_tmp")
                        nc.vector.tensor_tensor(v4(t), i0, i1, _mult)
                        nc.vector.tensor_tensor(dst[:], dst[:], t[:], _add)

            A2 = gp.tile([P, 25 * F], _bf16, tag="a2")
            matmul5(A2, Ab, Ab)
            A4 = gp.tile([P, 25 * F], _bf16, tag="a4")
            matmul5(A4, A2, A2)
            MM = gp.tile([P, 25 * F], _bf16, tag="mm")
            matmul5(MM, A4, Ab)

            # --- apply: out[k] = sum_j M[k,j] * fm[j], chunked over c ---
            for cc in range(NCH):
                c0 = cc * CCH
                fms = []
                for j in range(K):
                    t = fp.tile([P, FD], _bf16, tag=f"fm{j}")
                    nc.sync.dma_start(
                        out=t[:].rearrange("p (c f) -> p c f", c=CCH),
                        in_=fm[j, :, c0 : c0 + CCH, :],
                    )
                    fms.append(t)
                for k in range(K):
                    ot = op_.tile([P, FD], _bf16, tag="out")
                    o3 = ot[:].rearrange("p (c f) -> p c f", c=CCH)
                    for j in range(K):
                        mv = (
                            MM[:, (k * K + j) * F : (k * K + j + 1) * F]
                            .unsqueeze(1)
                            .broadcast_to((P, CCH, F))
                        )
                        f3 = fms[j][:].rearrange("p (c f) -> p c f", c=CCH)
                        if j == 0:
                            nc.vector.tensor_tensor(o3, f3, mv, _mult)
                        else:
                            t2 = tp.tile([P, FD], _bf16, tag="ap_tmp")
                            nc.vector.tensor_tensor(
                                t2[:].rearrange("p (c f) -> p c f", c=CCH),
                                f3,
                                mv,
                                _mult,
                            )
                            nc.vector.tensor_tensor(ot[:], ot[:], t2[:], _add)
                    nc.sync.dma_start(
                        out=out[k, :, c0 : c0 + CCH, :],
                        in_=o3,
                    )
    nc.finalize()
    return nc


def _get_nc():
    if "nc" not in _cache:
        _cache["nc"] = _build()
    return _cache["nc"]


def kernel(guidance, fm0, fm1, fm2, fm3, fm4):
    nc = _get_nc()
    fms = [np.asarray(x, dtype=np.float32) for x in (fm0, fm1, fm2, fm3, fm4)]
    guidance = np.asarray(guidance, dtype=np.float32)

    in_maps = []
    for s in range(NCORES):
        b, h0 = s // 2, (s % 2) * HSH
        # guidance: [25, HSH, W] -> [P, 25, F] (partition-major pixels)
        g_s = np.ascontiguousarray(
            guidance[b, :, h0 : h0 + HSH, :]
            .reshape(25, P, F)
            .transpose(1, 0, 2)
        )
        fm_s = np.empty((K, P, C, F), dtype=_npbf16)
        for j in range(K):
            fm_s[j] = (
                fms[j][b, :, h0 : h0 + HSH, :]
                .reshape(C, P, F)
                .transpose(1, 0, 2)
                .astype(_npbf16)
            )
        in_maps.append({"g": g_s, "fm": fm_s})

    res = run_bass_kernel_spmd(nc, in_maps, list(range(NCORES)))

    full = np.empty((K, B, C, H, W), dtype=np.float32)
    for s in range(NCORES):
        b, h0 = s // 2, (s % 2) * HSH
        o = res.results[s]["out"].astype(np.float32)  # [K, P, C, F]
        full[:, b, :, h0 : h0 + HSH, :] = o.transpose(0, 2, 1, 3).reshape(
            K, C, HSH, W
        )
    return full
